# revision 45
# baseline (speedup 1.0000x reference)
"""Multi-head attention forward on 8 Trainium2 NeuronCores (Bass/Tile).

Problem: B=2, S=2048, d_model=1024, 16 heads (depth 64), fp32.
  q/k/v = query @ W{q,k,v}; logits = q k^T / 8 + mask * -1e9;
  out = softmax(logits) v @ Wo.

Sharding (Megatron-style, hardcoded): core c handles batch b = c//4 and head
group hg = c%4 (4 heads = 256 of the 1024 head dims). Wq/Wk/Wv are
column-sharded, Wo row-sharded; each core emits a partial [S, 1024] output and
the host sums the 4 partials per batch (the "all-reduce").

Design: the roofline is ScalarE exp (16.8M elems ~ 1/cyc @1.2GHz = ~143us of
1024-wide activations); everything else is arranged to hide under it, and all
matmuls are bf16 (same PE rate as f32r, lower power -> less HAM throttling).

  * Only the g=0 head-group's q/k projections run before attention - d-major
    psum chains that ride the input DMA, so the first exp fires at ~23us.
  * "Sweep-offset AV": attention is four 16-iteration sweeps, one per head
    pair. Sweep s runs pair s's QK+exp+mask while the PE also runs pair
    s-1's AV accumulation (lagged a full sweep; masked weights are buffered
    in SBUF). Sweep 0 has no previous pair, so its PE slack runs the v
    projection and the g=1 q/k projection chains as fillers. This keeps
    PSUM at exactly 8 banks (2x psl + 2x pso) in every phase.
  * Softmax denominators are free: vt columns 0:64 are 1.0, so AV psum rows
    0:64 hold the denominator replicated (matmul cost depends only on N, not
    M). Epilogue = custom-DVE reciprocal straight off psum + one fused
    multiply writing normalized bf16 attnT. No PE transposes.
  * The (1-mask) multiply is a bf16 2x-mode DVE op; the mask is stored as
    two per-qcp tiles (32KB/partition each), the second prefetched mid-run.
  * Output projection is a short tail; results go out as fp16 (host upcasts
    and batch-sums in fp32), halving output DMA.
"""

import sys

import numpy as np

sys.path.insert(0, "/opt/trn_rl_repo")

B = 2
S = 2048
D = 1024
HEADS = 16
DEPTH = 64
CORES = 8
HG = 4          # head groups (cores per batch)
HPC = 4         # heads per core
DH = HPC * DEPTH  # per-core head width = 256

_CACHE = {}
DEBUG = False


def _build_program():
    import concourse.bass as bass  # noqa: F401  (registers engines)
    import concourse.mybir as mybir
    import concourse.tile as tile
    from concourse import bacc
    from concourse.bass_interp import get_hw_module
    from concourse.masks import make_identity

    dt = mybir.dt
    f32, bf16, fp16 = dt.float32, dt.bfloat16, dt.float16
    MULT = mybir.AluOpType.mult
    EXP = mybir.ActivationFunctionType.Exp

    nc = bacc.Bacc(
        "TRN2",
        target_bir_lowering=False,
        debug=False,
        enable_asserts=True,
        num_devices=CORES,
    )

    xT = nc.dram_tensor("xT", [D, S], bf16, kind="ExternalInput").ap()
    imaskT = nc.dram_tensor("imaskT", [S, S], bf16, kind="ExternalInput").ap()
    wq = nc.dram_tensor("wq", [D, DH], bf16, kind="ExternalInput").ap()
    wk = nc.dram_tensor("wk", [D, DH], bf16, kind="ExternalInput").ap()
    wv = nc.dram_tensor("wv", [D, DH], bf16, kind="ExternalInput").ap()
    wo = nc.dram_tensor("wo", [DH, D], bf16, kind="ExternalInput").ap()
    out = nc.dram_tensor("out", [S, D], fp16, kind="ExternalOutput").ap()
    if DEBUG:
        dbg_qT = nc.dram_tensor("dbg_qT", [2, 128, S], bf16, kind="ExternalOutput").ap()
        dbg_kT = nc.dram_tensor("dbg_kT", [2, 128, S], bf16, kind="ExternalOutput").ap()
        dbg_vt0 = nc.dram_tensor("dbg_vt0", [128, HPC, 128], bf16, kind="ExternalOutput").ap()
        dbg_attnT = nc.dram_tensor("dbg_attnT", [2, 128, S], bf16, kind="ExternalOutput").ap()
        dbg_rden = nc.dram_tensor("dbg_rden", [64, 1024], f32, kind="ExternalOutput").ap()
        dbg_num = nc.dram_tensor("dbg_num", [64, 1024], f32, kind="ExternalOutput").ap()

    with tile.TileContext(nc) as tc:
        with tc.tile_pool(name="persist", bufs=1) as pp:
            # Persistent SBUF tiles.
            qT = [pp.tile([128, S], bf16, tag=f"qT{g}", name=f"qT{g}") for g in range(2)]
            kT = [pp.tile([128, S], bf16, tag=f"kT{g}", name=f"kT{g}") for g in range(2)]
            # vt: per 128-row k-block, per head: cols 0:64 = 1.0 (denominator
            # rows of the AV psum), cols 64:128 = V.
            vt = [pp.tile([128, HPC, 128], bf16, tag=f"v{i}", name=f"v{i}") for i in range(16)]
            wot = [pp.tile([128, D], bf16, tag=f"wo{g}", name=f"wo{g}") for g in range(2)]
            attnT = [pp.tile([128, S], bf16, tag=f"attnT{g}", name=f"attnT{g}") for g in range(2)]
            # mask for qcp0; the qcp1 tile is allocated after the x/w pool
            # frees (SBUF headroom) and prefetched during sweep 1.
            mt = [pp.tile([128, 16, 1024], bf16, tag="mask0", name="mask0"), None]
            ident = pp.tile([128, 128], f32, tag="ident", name="ident")
            identb = pp.tile([128, 128], bf16, tag="identb", name="identb")

            make_identity(nc, ident[:])
            nc.vector.tensor_copy(identb[:], ident[:])
            for st in range(16):
                nc.gpsimd.memset(vt[st][:, :, 0:DEPTH], 1.0)

            # HAM warmup so the PE is at speed when projections fire.
            with tc.tile_pool(name="psW", bufs=2, space="PSUM") as psW:
                for w in range(16):
                    psw = psW.tile([128, 128], f32, tag="warm", name="warm")
                    nc.tensor.matmul(psw[:], ident[:], ident[:],
                                     start=True, stop=True)

            imaskT_r = imaskT.rearrange("(t p) q -> p t q", p=128)

            with tc.tile_pool(name="exs", bufs=3) as exs, \
                 tc.tile_pool(name="eps", bufs=2) as eps, \
                 tc.tile_pool(name="psL", bufs=2, space="PSUM") as psL:
              with tc.tile_pool(name="xw", bufs=1) as xw:
                xt = [xw.tile([128, S], bf16, tag=f"x{d}", name=f"x{d}") for d in range(8)]
                wts = {}
                for nm in ("wq", "wk", "wv"):
                    wts[nm] = [xw.tile([128, DH], bf16, tag=f"{nm}{d}", name=f"{nm}{d}") for d in range(8)]
                # d-interleaved input DMAs, then qcp0 mask chunks, wo.
                for d in range(8):
                    nc.sync.dma_start(wts["wq"][d][:], wq[d * 128:(d + 1) * 128, :])
                    nc.sync.dma_start(wts["wk"][d][:], wk[d * 128:(d + 1) * 128, :])
                    nc.sync.dma_start(wts["wv"][d][:], wv[d * 128:(d + 1) * 128, :])
                    nc.sync.dma_start(xt[d][:], xT[d * 128:(d + 1) * 128, :])
                for kb in range(16):
                    nc.sync.dma_start(mt[0][:, kb:kb + 1, :],
                                      imaskT_r[:, kb:kb + 1, 0:1024])
                for g in range(2):
                    nc.sync.dma_start(wot[g][:], wo[g * 128:(g + 1) * 128, :])

                # ---- g=0 q/k projections: d-major waves over 8 psum banks,
                # riding the input DMA. ScalarE drains (it is idle here).
                with tc.tile_pool(name="psA", bufs=2, space="PSUM") as psA:
                    for wt, dst in ((wts["wq"], qT), (wts["wk"], kT)):
                        chains = [(sh, psA.tile([128, 1024], f32, tag="proj", name="proj"))
                                  for sh in range(2)]
                        for d in range(8):
                            for sh, ps in chains:
                                for half in range(2):
                                    hs = slice(half * 512, (half + 1) * 512)
                                    nc.tensor.matmul(
                                        ps[:, hs],
                                        wt[d][:, 0:128],
                                        xt[d][:, sh * 1024 + half * 512:
                                               sh * 1024 + half * 512 + 512],
                                        start=(d == 0), stop=(d == 7),
                                    )
                        for sh, ps in chains:
                            nc.scalar.copy(dst[0][:, sh * 1024:(sh + 1) * 1024], ps[:])

                # ---- attention: 4 sweep-offset pairs ----
                PAIRS = [(0, 0), (0, 1), (1, 0), (1, 1)]   # (qcp, g)
                emq = {}      # (pair_idx, h, kb) -> em tile
                psoq = {}     # pair_idx -> [pso_h0, pso_h1]

                def qk_exp(p, kb):
                    qcp, g = PAIRS[p]
                    # halves interleaved h0,h1,h0,h1: the two heads sit in
                    # disjoint PE row groups (rows 0:64 / 64:128), so adjacent
                    # matmuls run concurrently - half the QK wall time.
                    psl = [psL.tile([128, 1024], f32, tag="lg", name="lg")
                           for _ in range(2)]
                    for half in range(2):
                        hs = slice(half * 512, (half + 1) * 512)
                        qh = slice(qcp * 1024 + half * 512,
                                   qcp * 1024 + half * 512 + 512)
                        for h in range(2):
                            po = h * 64
                            nc.tensor.matmul(
                                psl[h][:, hs],
                                kT[g][po:po + 64, kb * 128:(kb + 1) * 128],
                                qT[g][po:po + 64, qh],
                                start=True, stop=True,
                            )
                    for h in range(2):
                        ex = exs.tile([128, 1024], bf16, tag="ex", name="ex")
                        nc.scalar.activation(ex[:], psl[h][:], EXP, scale=0.125)
                        em = exs.tile([128, 1024], bf16, tag="em", name="em", bufs=31)
                        nc.vector.tensor_tensor(em[:], ex[:], mt[qcp][:, kb, :], MULT)
                        emq[(p, h, kb)] = em

                def av(p, kb, psO):
                    qcp, g = PAIRS[p]
                    if kb == 0:
                        psoq[p] = [psO.tile([128, 1024], f32, tag="av", name=f"av{h}")
                                   for h in range(2)]
                    for h in range(2):
                        for half in range(2):
                            hs = slice(half * 512, (half + 1) * 512)
                            nc.tensor.matmul(
                                psoq[p][h][:, hs],
                                vt[kb][:, 2 * g + h, :],
                                emq[(p, h, kb)][:, hs],
                                start=(kb == 0), stop=(kb == 15),
                            )
                        if kb == 15:
                            for kk in range(16):
                                del emq[(p, h, kk)]

                def epilogue(p):
                    qcp, g = PAIRS[p]
                    qs = slice(qcp * 1024, (qcp + 1) * 1024)
                    for h in range(2):
                        po = h * 64
                        pso = psoq[p][h]
                        rden = eps.tile([64, 1024], f32, tag="rden", name="rden")
                        nc.vector.reciprocal_approx_fast(rden[:], pso[0:64, :])
                        if DEBUG and p == 0 and h == 0:
                            num_s = eps.tile([64, 1024], f32, tag="nums", name="nums")
                            nc.vector.tensor_copy(num_s[:], pso[64:128, :])
                            nc.sync.dma_start(dbg_num[:], num_s[:])
                            nc.sync.dma_start(dbg_rden[:], rden[:])
                        nc.vector.tensor_tensor(
                            attnT[g][po:po + 64, qs],
                            pso[64:128, :], rden[:], MULT,
                        )

                # Sweep 0: pair 0 QK/exp with v-proj + g=1 q/k proj as PE
                # fillers (no AVs yet -> psum banks 5-8 are free for them).
                with tc.tile_pool(name="psV", bufs=2, space="PSUM") as psV, \
                     tc.tile_pool(name="psA2", bufs=2, space="PSUM") as psA2:
                    g1chains = []
                    for wt, dst in ((wts["wq"], qT), (wts["wk"], kT)):
                        for sc in range(4):
                            g1chains.append((wt, dst, sc))
                    g1state = {}

                    def g1_step(kb):
                        # half a chain (4 d-matmuls) per iteration
                        ci = kb // 2
                        if ci >= len(g1chains):
                            return
                        wt, dst, sc = g1chains[ci]
                        if kb % 2 == 0:
                            g1state[ci] = psA2.tile([128, 512], f32, tag="g1", name="g1")
                            for d in range(4):
                                nc.tensor.matmul(
                                    g1state[ci][:],
                                    wt[d][:, 128:256],
                                    xt[d][:, sc * 512:(sc + 1) * 512],
                                    start=(d == 0), stop=False,
                                )
                        else:
                            for d in range(4, 8):
                                nc.tensor.matmul(
                                    g1state[ci][:],
                                    wt[d][:, 128:256],
                                    xt[d][:, sc * 512:(sc + 1) * 512],
                                    start=False, stop=(d == 7),
                                )
                            nc.vector.tensor_copy(
                                dst[1][:, sc * 512:(sc + 1) * 512], g1state[ci][:])

                    def v_chain(st):
                        ps = psV.tile([128, DH], f32, tag="vproj", name="vproj")
                        for d in range(8):
                            nc.tensor.matmul(
                                ps[:],
                                xt[d][:, st * 128:(st + 1) * 128],
                                wts["wv"][d][:],
                                start=(d == 0), stop=(d == 7),
                            )
                        nc.vector.tensor_copy(
                            vt[st][:, :, DEPTH:128],
                            ps[:].rearrange("p (h e) -> p h e", h=HPC),
                        )

                    for kb in range(16):
                        qk_exp(0, kb)
                        v_chain(kb)
                        g1_step(kb)

              # x/w pool closed: allocate the qcp1 mask tile and prefetch it
              # during sweep 1.
              with tc.tile_pool(name="mt1", bufs=1) as mp:
                mt[1] = mp.tile([128, 16, 1024], bf16, tag="mask1", name="mask1")
                for kb in range(16):
                    nc.sync.dma_start(mt[1][:, kb:kb + 1, :],
                                      imaskT_r[:, kb:kb + 1, 1024:2048])

                # Sweeps 1-3 + final AV burst.
                with tc.tile_pool(name="psO", bufs=2, space="PSUM") as psO:
                    for p in range(1, 4):
                        for _ in range(4):
                            nc.tensor.ldweights(identb[:])
                        for kb in range(16):
                            qk_exp(p, kb)
                            av(p - 1, kb, psO)
                        epilogue(p - 1)
                    for kb in range(16):
                        av(3, kb, psO)
                    epilogue(3)

            if DEBUG:
                for g in range(2):
                    nc.sync.dma_start(dbg_qT[g], qT[g][:])
                    nc.sync.dma_start(dbg_kT[g], kT[g][:])
                    nc.sync.dma_start(dbg_attnT[g], attnT[g][:])
                nc.sync.dma_start(dbg_vt0[:], vt[0][:])

            # ---- output projection tail (fp16 out) ----
            with tc.tile_pool(name="ot", bufs=3) as ob, \
                 tc.tile_pool(name="psF", bufs=2, space="PSUM") as psF:
                for st in range(16):
                    psf = psF.tile([128, D], f32, tag="po", name="po")
                    for nch in range(2):
                        hs = slice(nch * 512, (nch + 1) * 512)
                        for g in range(2):
                            nc.tensor.matmul(
                                psf[:, hs],
                                attnT[g][:, st * 128:(st + 1) * 128],
                                wot[g][:, hs],
                                start=(g == 0), stop=(g == 1),
                            )
                    ot = ob.tile([128, D], fp16, tag="otile", name="otile")
                    if st % 2 == 0:
                        nc.vector.tensor_copy(ot[:], psf[:])
                    else:
                        nc.scalar.copy(ot[:], psf[:])
                    nc.sync.dma_start(out[st * 128:(st + 1) * 128, :], ot[:])

    nc.compile()
    nc.m = get_hw_module(nc.m)
    return nc


def _get_program():
    if "nc" not in _CACHE:
        _CACHE["nc"] = _build_program()
    return _CACHE["nc"]


def _make_in_maps(query, attention_mask, Wq, Wk, Wv, Wo):
    import ml_dtypes

    bf16 = ml_dtypes.bfloat16
    in_maps = []
    imaskT_b = []
    xT_b = []
    for b in range(B):
        imaskT_b.append(
            np.ascontiguousarray(1 - attention_mask[b, 0].T).astype(bf16)
        )
        xT_b.append(np.ascontiguousarray(query[b].T.astype(bf16)))
    for c in range(CORES):
        b, hg = c // HG, c % HG
        cs = slice(hg * DH, (hg + 1) * DH)
        in_maps.append({
            "xT": xT_b[b],
            "imaskT": imaskT_b[b],
            "wq": np.ascontiguousarray(Wq[:, cs].astype(bf16)),
            "wk": np.ascontiguousarray(Wk[:, cs].astype(bf16)),
            "wv": np.ascontiguousarray(Wv[:, cs].astype(bf16)),
            "wo": np.ascontiguousarray(Wo[cs, :].astype(bf16)),
        })
    return in_maps


def _run(inputs, trace=False):
    from concourse.bass_utils import run_bass_kernel_spmd

    nc = _get_program()
    in_maps = _make_in_maps(**inputs)
    res = run_bass_kernel_spmd(
        nc, in_maps, core_ids=list(range(CORES)), trace=trace,
    )
    outs = [res.results[c]["out"].astype(np.float32) for c in range(CORES)]
    full = np.empty((B, S, D), dtype=np.float32)
    for b in range(B):
        acc = outs[4 * b]
        for hg in range(1, HG):
            acc = acc + outs[4 * b + hg]
        full[b] = acc
    return full, res


def kernel(query, attention_mask, Wq, Wk, Wv, Wo):
    full, _ = _run(dict(
        query=np.asarray(query), attention_mask=np.asarray(attention_mask),
        Wq=np.asarray(Wq), Wk=np.asarray(Wk), Wv=np.asarray(Wv),
        Wo=np.asarray(Wo),
    ))
    return full


# revision 46
# speedup vs baseline: 1.0529x; 1.0529x over previous
"""Multi-head attention forward on 8 Trainium2 NeuronCores (Bass/Tile).

Problem: B=2, S=2048, d_model=1024, 16 heads (depth 64), fp32.
  q/k/v = query @ W{q,k,v}; logits = q k^T / 8 + mask * -1e9;
  out = softmax(logits) v @ Wo.

Sharding (Megatron-style, hardcoded): core c handles batch b = c//4 and head
group hg = c%4 (4 heads = 256 of the 1024 head dims). Wq/Wk/Wv are
column-sharded, Wo row-sharded; each core emits a partial [S, 1024] output and
the host sums the 4 partials per batch (the "all-reduce").

Design: the roofline is ScalarE exp (16.8M elems ~ 1/cyc @1.2GHz = ~143us of
1024-wide activations); everything else is arranged to hide under it, and all
matmuls are bf16 (same PE rate as f32r, lower power -> less HAM throttling).

  * Only the g=0 head-group's q/k projections run before attention - d-major
    psum chains that ride the input DMA, so the first exp fires at ~23us.
  * "Sweep-offset AV": attention is four 16-iteration sweeps, one per head
    pair. Sweep s runs pair s's QK+exp+mask while the PE also runs pair
    s-1's AV accumulation (lagged a full sweep; masked weights are buffered
    in SBUF). Sweep 0 has no previous pair, so its PE slack runs the v
    projection and the g=1 q/k projection chains as fillers. This keeps
    PSUM at exactly 8 banks (2x psl + 2x pso) in every phase.
  * Softmax denominators are free: vt columns 0:64 are 1.0, so AV psum rows
    0:64 hold the denominator replicated (matmul cost depends only on N, not
    M). Epilogue = custom-DVE reciprocal straight off psum + one fused
    multiply writing normalized bf16 attnT. No PE transposes.
  * The (1-mask) multiply is a bf16 2x-mode DVE op; the mask is stored as
    two per-qcp tiles (32KB/partition each), the second prefetched mid-run.
  * Output projection is a short tail; results go out as fp16 (host upcasts
    and batch-sums in fp32), halving output DMA.
"""

import sys

import numpy as np

sys.path.insert(0, "/opt/trn_rl_repo")

B = 2
S = 2048
D = 1024
HEADS = 16
DEPTH = 64
CORES = 8
HG = 4          # head groups (cores per batch)
HPC = 4         # heads per core
DH = HPC * DEPTH  # per-core head width = 256

_CACHE = {}
DEBUG = False


def _build_program():
    import concourse.bass as bass  # noqa: F401  (registers engines)
    import concourse.mybir as mybir
    import concourse.tile as tile
    from concourse import bacc
    from concourse.bass_interp import get_hw_module
    from concourse.masks import make_identity

    dt = mybir.dt
    f32, bf16, fp16 = dt.float32, dt.bfloat16, dt.float16
    MULT = mybir.AluOpType.mult
    EXP = mybir.ActivationFunctionType.Exp

    nc = bacc.Bacc(
        "TRN2",
        target_bir_lowering=False,
        debug=False,
        enable_asserts=True,
        num_devices=CORES,
    )

    xT = nc.dram_tensor("xT", [D, S], bf16, kind="ExternalInput").ap()
    imaskT = nc.dram_tensor("imaskT", [S, S], bf16, kind="ExternalInput").ap()
    wq = nc.dram_tensor("wq", [D, DH], bf16, kind="ExternalInput").ap()
    wk = nc.dram_tensor("wk", [D, DH], bf16, kind="ExternalInput").ap()
    wv = nc.dram_tensor("wv", [D, DH], bf16, kind="ExternalInput").ap()
    wo = nc.dram_tensor("wo", [DH, D], bf16, kind="ExternalInput").ap()
    out = nc.dram_tensor("out", [S, D], fp16, kind="ExternalOutput").ap()
    if DEBUG:
        dbg_qT = nc.dram_tensor("dbg_qT", [2, 128, S], bf16, kind="ExternalOutput").ap()
        dbg_kT = nc.dram_tensor("dbg_kT", [2, 128, S], bf16, kind="ExternalOutput").ap()
        dbg_vt0 = nc.dram_tensor("dbg_vt0", [128, HPC, 128], bf16, kind="ExternalOutput").ap()
        dbg_attnT = nc.dram_tensor("dbg_attnT", [2, 128, S], bf16, kind="ExternalOutput").ap()
        dbg_rden = nc.dram_tensor("dbg_rden", [64, 1024], f32, kind="ExternalOutput").ap()
        dbg_num = nc.dram_tensor("dbg_num", [64, 1024], f32, kind="ExternalOutput").ap()

    with tile.TileContext(nc) as tc:
        with tc.tile_pool(name="persist", bufs=1) as pp:
            # Persistent SBUF tiles.
            qT = [pp.tile([128, S], bf16, tag=f"qT{g}", name=f"qT{g}") for g in range(2)]
            kT = [pp.tile([128, S], bf16, tag=f"kT{g}", name=f"kT{g}") for g in range(2)]
            # vt: per 128-row k-block, per head: cols 0:64 = 1.0 (denominator
            # rows of the AV psum), cols 64:128 = V.
            vt = [pp.tile([128, HPC, 128], bf16, tag=f"v{i}", name=f"v{i}") for i in range(16)]
            wot = [pp.tile([128, D], bf16, tag=f"wo{g}", name=f"wo{g}") for g in range(2)]
            attnT = [pp.tile([128, S], bf16, tag=f"attnT{g}", name=f"attnT{g}") for g in range(2)]
            # mask for qcp0; the qcp1 tile is allocated after the x/w pool
            # frees (SBUF headroom) and prefetched during sweep 1.
            mt = [pp.tile([128, 16, 1024], bf16, tag="mask0", name="mask0"), None]
            ident = pp.tile([128, 128], f32, tag="ident", name="ident")
            identb = pp.tile([128, 128], bf16, tag="identb", name="identb")

            make_identity(nc, ident[:])
            nc.vector.tensor_copy(identb[:], ident[:])
            for st in range(16):
                nc.gpsimd.memset(vt[st][:, :, 0:DEPTH], 1.0)

            # HAM warmup so the PE is at speed when projections fire.
            with tc.tile_pool(name="psW", bufs=2, space="PSUM") as psW:
                for w in range(16):
                    psw = psW.tile([128, 128], f32, tag="warm", name="warm")
                    nc.tensor.matmul(psw[:], ident[:], ident[:],
                                     start=True, stop=True)

            imaskT_r = imaskT.rearrange("(t p) q -> p t q", p=128)

            with tc.tile_pool(name="exs", bufs=3) as exs, \
                 tc.tile_pool(name="eps", bufs=2) as eps, \
                 tc.tile_pool(name="psL", bufs=2, space="PSUM") as psL:
              with tc.tile_pool(name="xw", bufs=1) as xw:
                xt = [xw.tile([128, S], bf16, tag=f"x{d}", name=f"x{d}") for d in range(8)]
                wts = {}
                for nm in ("wq", "wk", "wv"):
                    wts[nm] = [xw.tile([128, DH], bf16, tag=f"{nm}{d}", name=f"{nm}{d}") for d in range(8)]
                # DMA priority order: xt+wq feed the q chains immediately;
                # wk is needed only when the k chains start (~18us), wv and
                # the mask only at sweep 0, wo only at the tail.
                for d in range(8):
                    nc.sync.dma_start(wts["wq"][d][:], wq[d * 128:(d + 1) * 128, :])
                    nc.sync.dma_start(xt[d][:], xT[d * 128:(d + 1) * 128, :])
                for d in range(8):
                    nc.sync.dma_start(wts["wk"][d][:], wk[d * 128:(d + 1) * 128, :])
                for d in range(8):
                    nc.sync.dma_start(wts["wv"][d][:], wv[d * 128:(d + 1) * 128, :])
                for kb in range(16):
                    nc.sync.dma_start(mt[0][:, kb:kb + 1, :],
                                      imaskT_r[:, kb:kb + 1, 0:1024])
                for g in range(2):
                    nc.sync.dma_start(wot[g][:], wo[g * 128:(g + 1) * 128, :])

                # ---- g=0 q/k projections: d-major waves over 8 psum banks,
                # riding the input DMA. ScalarE drains (it is idle here).
                with tc.tile_pool(name="psA", bufs=2, space="PSUM") as psA:
                    for wt, dst, drain in ((wts["wq"], qT, "dve"),
                                           (wts["wk"], kT, "act")):
                        chains = [(sh, psA.tile([128, 1024], f32, tag="proj", name="proj"))
                                  for sh in range(2)]
                        for d in range(8):
                            for sh, ps in chains:
                                for half in range(2):
                                    hs = slice(half * 512, (half + 1) * 512)
                                    nc.tensor.matmul(
                                        ps[:, hs],
                                        wt[d][:, 0:128],
                                        xt[d][:, sh * 1024 + half * 512:
                                               sh * 1024 + half * 512 + 512],
                                        start=(d == 0), stop=(d == 7),
                                    )
                        for sh, ps in chains:
                            if drain == "dve":
                                nc.vector.tensor_copy(dst[0][:, sh * 1024:(sh + 1) * 1024], ps[:])
                            else:
                                nc.scalar.copy(dst[0][:, sh * 1024:(sh + 1) * 1024], ps[:])

                # ---- attention: 4 sweep-offset pairs ----
                PAIRS = [(0, 0), (0, 1), (1, 0), (1, 1)]   # (qcp, g)
                emq = {}      # (pair_idx, h, kb) -> em tile
                psoq = {}     # pair_idx -> [pso_h0, pso_h1]

                def qk_exp(p, kb):
                    qcp, g = PAIRS[p]
                    # halves interleaved h0,h1,h0,h1: the two heads sit in
                    # disjoint PE row groups (rows 0:64 / 64:128), so adjacent
                    # matmuls run concurrently - half the QK wall time.
                    psl = [psL.tile([128, 1024], f32, tag="lg", name="lg")
                           for _ in range(2)]
                    for half in range(2):
                        hs = slice(half * 512, (half + 1) * 512)
                        qh = slice(qcp * 1024 + half * 512,
                                   qcp * 1024 + half * 512 + 512)
                        for h in range(2):
                            po = h * 64
                            nc.tensor.matmul(
                                psl[h][:, hs],
                                kT[g][po:po + 64, kb * 128:(kb + 1) * 128],
                                qT[g][po:po + 64, qh],
                                start=True, stop=True,
                            )
                    for h in range(2):
                        ex = exs.tile([128, 1024], bf16, tag="ex", name="ex")
                        nc.scalar.activation(ex[:], psl[h][:], EXP, scale=0.125)
                        em = exs.tile([128, 1024], bf16, tag="em", name="em", bufs=31)
                        nc.vector.tensor_tensor(em[:], ex[:], mt[qcp][:, kb, :], MULT)
                        emq[(p, h, kb)] = em

                def av(p, kb, psO):
                    qcp, g = PAIRS[p]
                    if kb == 0:
                        psoq[p] = [psO.tile([128, 1024], f32, tag="av", name=f"av{h}")
                                   for h in range(2)]
                    for h in range(2):
                        for half in range(2):
                            hs = slice(half * 512, (half + 1) * 512)
                            nc.tensor.matmul(
                                psoq[p][h][:, hs],
                                vt[kb][:, 2 * g + h, :],
                                emq[(p, h, kb)][:, hs],
                                start=(kb == 0), stop=(kb == 15),
                            )
                        if kb == 15:
                            for kk in range(16):
                                del emq[(p, h, kk)]

                def epilogue(p):
                    qcp, g = PAIRS[p]
                    qs = slice(qcp * 1024, (qcp + 1) * 1024)
                    for h in range(2):
                        po = h * 64
                        pso = psoq[p][h]
                        rden = eps.tile([64, 1024], f32, tag="rden", name="rden")
                        nc.vector.reciprocal_approx_fast(rden[:], pso[0:64, :])
                        if DEBUG and p == 0 and h == 0:
                            num_s = eps.tile([64, 1024], f32, tag="nums", name="nums")
                            nc.vector.tensor_copy(num_s[:], pso[64:128, :])
                            nc.sync.dma_start(dbg_num[:], num_s[:])
                            nc.sync.dma_start(dbg_rden[:], rden[:])
                        nc.vector.tensor_tensor(
                            attnT[g][po:po + 64, qs],
                            pso[64:128, :], rden[:], MULT,
                        )

                # Sweep 0: pair 0 QK/exp with v-proj + g=1 q/k proj as PE
                # fillers (no AVs yet -> psum banks 5-8 are free for them).
                with tc.tile_pool(name="psV", bufs=2, space="PSUM") as psV, \
                     tc.tile_pool(name="psA2", bufs=2, space="PSUM") as psA2:
                    g1chains = []
                    for wt, dst in ((wts["wq"], qT), (wts["wk"], kT)):
                        for sc in range(4):
                            g1chains.append((wt, dst, sc))
                    g1state = {}

                    def g1_step(kb):
                        # half a chain (4 d-matmuls) per iteration
                        ci = kb // 2
                        if ci >= len(g1chains):
                            return
                        wt, dst, sc = g1chains[ci]
                        if kb % 2 == 0:
                            g1state[ci] = psA2.tile([128, 512], f32, tag="g1", name="g1")
                            for d in range(4):
                                nc.tensor.matmul(
                                    g1state[ci][:],
                                    wt[d][:, 128:256],
                                    xt[d][:, sc * 512:(sc + 1) * 512],
                                    start=(d == 0), stop=False,
                                )
                        else:
                            for d in range(4, 8):
                                nc.tensor.matmul(
                                    g1state[ci][:],
                                    wt[d][:, 128:256],
                                    xt[d][:, sc * 512:(sc + 1) * 512],
                                    start=False, stop=(d == 7),
                                )
                            nc.vector.tensor_copy(
                                dst[1][:, sc * 512:(sc + 1) * 512], g1state[ci][:])

                    def v_chain(st):
                        ps = psV.tile([128, DH], f32, tag="vproj", name="vproj")
                        for d in range(8):
                            nc.tensor.matmul(
                                ps[:],
                                xt[d][:, st * 128:(st + 1) * 128],
                                wts["wv"][d][:],
                                start=(d == 0), stop=(d == 7),
                            )
                        nc.vector.tensor_copy(
                            vt[st][:, :, DEPTH:128],
                            ps[:].rearrange("p (h e) -> p h e", h=HPC),
                        )

                    for kb in range(16):
                        qk_exp(0, kb)
                        v_chain(kb)
                        g1_step(kb)

              # x/w pool closed: allocate the qcp1 mask tile and prefetch it
              # during sweep 1.
              with tc.tile_pool(name="mt1", bufs=1) as mp:
                mt[1] = mp.tile([128, 16, 1024], bf16, tag="mask1", name="mask1")
                for kb in range(16):
                    nc.sync.dma_start(mt[1][:, kb:kb + 1, :],
                                      imaskT_r[:, kb:kb + 1, 1024:2048])

                # Sweeps 1-3 + final AV burst.
                with tc.tile_pool(name="psO", bufs=2, space="PSUM") as psO:
                    for p in range(1, 4):
                        for _ in range(4):
                            nc.tensor.ldweights(identb[:])
                        for kb in range(16):
                            qk_exp(p, kb)
                            av(p - 1, kb, psO)
                        epilogue(p - 1)
                    for kb in range(16):
                        av(3, kb, psO)
                    epilogue(3)

            if DEBUG:
                for g in range(2):
                    nc.sync.dma_start(dbg_qT[g], qT[g][:])
                    nc.sync.dma_start(dbg_kT[g], kT[g][:])
                    nc.sync.dma_start(dbg_attnT[g], attnT[g][:])
                nc.sync.dma_start(dbg_vt0[:], vt[0][:])

            # ---- output projection tail (fp16 out) ----
            with tc.tile_pool(name="ot", bufs=3) as ob, \
                 tc.tile_pool(name="psF", bufs=2, space="PSUM") as psF:
                for st in range(16):
                    psf = psF.tile([128, D], f32, tag="po", name="po")
                    for nch in range(2):
                        hs = slice(nch * 512, (nch + 1) * 512)
                        for g in range(2):
                            nc.tensor.matmul(
                                psf[:, hs],
                                attnT[g][:, st * 128:(st + 1) * 128],
                                wot[g][:, hs],
                                start=(g == 0), stop=(g == 1),
                            )
                    ot = ob.tile([128, D], fp16, tag="otile", name="otile")
                    if st % 2 == 0:
                        nc.vector.tensor_copy(ot[:], psf[:])
                    else:
                        nc.scalar.copy(ot[:], psf[:])
                    nc.sync.dma_start(out[st * 128:(st + 1) * 128, :], ot[:])

    nc.compile()
    nc.m = get_hw_module(nc.m)
    return nc


def _get_program():
    if "nc" not in _CACHE:
        _CACHE["nc"] = _build_program()
    return _CACHE["nc"]


def _make_in_maps(query, attention_mask, Wq, Wk, Wv, Wo):
    import ml_dtypes

    bf16 = ml_dtypes.bfloat16
    in_maps = []
    imaskT_b = []
    xT_b = []
    for b in range(B):
        imaskT_b.append(
            np.ascontiguousarray(1 - attention_mask[b, 0].T).astype(bf16)
        )
        xT_b.append(np.ascontiguousarray(query[b].T.astype(bf16)))
    for c in range(CORES):
        b, hg = c // HG, c % HG
        cs = slice(hg * DH, (hg + 1) * DH)
        in_maps.append({
            "xT": xT_b[b],
            "imaskT": imaskT_b[b],
            "wq": np.ascontiguousarray(Wq[:, cs].astype(bf16)),
            "wk": np.ascontiguousarray(Wk[:, cs].astype(bf16)),
            "wv": np.ascontiguousarray(Wv[:, cs].astype(bf16)),
            "wo": np.ascontiguousarray(Wo[cs, :].astype(bf16)),
        })
    return in_maps


def _run(inputs, trace=False):
    from concourse.bass_utils import run_bass_kernel_spmd

    nc = _get_program()
    in_maps = _make_in_maps(**inputs)
    res = run_bass_kernel_spmd(
        nc, in_maps, core_ids=list(range(CORES)), trace=trace,
    )
    outs = [res.results[c]["out"].astype(np.float32) for c in range(CORES)]
    full = np.empty((B, S, D), dtype=np.float32)
    for b in range(B):
        acc = outs[4 * b]
        for hg in range(1, HG):
            acc = acc + outs[4 * b + hg]
        full[b] = acc
    return full, res


def kernel(query, attention_mask, Wq, Wk, Wv, Wo):
    full, _ = _run(dict(
        query=np.asarray(query), attention_mask=np.asarray(attention_mask),
        Wq=np.asarray(Wq), Wk=np.asarray(Wk), Wv=np.asarray(Wv),
        Wo=np.asarray(Wo),
    ))
    return full


# revision 47
# speedup vs baseline: 1.0778x; 1.0236x over previous
"""Multi-head attention forward on 8 Trainium2 NeuronCores (Bass/Tile).

Problem: B=2, S=2048, d_model=1024, 16 heads (depth 64), fp32.
  q/k/v = query @ W{q,k,v}; logits = q k^T / 8 + mask * -1e9;
  out = softmax(logits) v @ Wo.

Sharding (Megatron-style, hardcoded): core c handles batch b = c//4 and head
group hg = c%4 (4 heads = 256 of the 1024 head dims). Wq/Wk/Wv are
column-sharded, Wo row-sharded; each core emits a partial [S, 1024] output and
the host sums the 4 partials per batch (the "all-reduce").

Design: the roofline is ScalarE exp (16.8M elems ~ 1/cyc @1.2GHz = ~143us of
1024-wide activations); everything else is arranged to hide under it, and all
matmuls are bf16 (same PE rate as f32r, lower power -> less HAM throttling).

  * Only the g=0 head-group's q/k projections run before attention - d-major
    psum chains that ride the input DMA, so the first exp fires at ~23us.
  * "Sweep-offset AV": attention is four 16-iteration sweeps, one per head
    pair. Sweep s runs pair s's QK+exp+mask while the PE also runs pair
    s-1's AV accumulation (lagged a full sweep; masked weights are buffered
    in SBUF). Sweep 0 has no previous pair, so its PE slack runs the v
    projection and the g=1 q/k projection chains as fillers. This keeps
    PSUM at exactly 8 banks (2x psl + 2x pso) in every phase.
  * Softmax denominators are free: vt columns 0:64 are 1.0, so AV psum rows
    0:64 hold the denominator replicated (matmul cost depends only on N, not
    M). Epilogue = custom-DVE reciprocal straight off psum + one fused
    multiply writing normalized bf16 attnT. No PE transposes.
  * The (1-mask) multiply is a bf16 2x-mode DVE op; the mask is stored as
    two per-qcp tiles (32KB/partition each), the second prefetched mid-run.
  * Output projection is a short tail; results go out as fp16 (host upcasts
    and batch-sums in fp32), halving output DMA.
"""

import sys

import numpy as np

sys.path.insert(0, "/opt/trn_rl_repo")

B = 2
S = 2048
D = 1024
HEADS = 16
DEPTH = 64
CORES = 8
HG = 4          # head groups (cores per batch)
HPC = 4         # heads per core
DH = HPC * DEPTH  # per-core head width = 256

_CACHE = {}
DEBUG = False


def _build_program():
    import concourse.bass as bass  # noqa: F401  (registers engines)
    import concourse.mybir as mybir
    import concourse.tile as tile
    from concourse import bacc
    from concourse.bass_interp import get_hw_module
    from concourse.masks import make_identity

    dt = mybir.dt
    f32, bf16, fp16 = dt.float32, dt.bfloat16, dt.float16
    MULT = mybir.AluOpType.mult
    EXP = mybir.ActivationFunctionType.Exp

    nc = bacc.Bacc(
        "TRN2",
        target_bir_lowering=False,
        debug=False,
        enable_asserts=True,
        num_devices=CORES,
    )

    xT = nc.dram_tensor("xT", [D, S], bf16, kind="ExternalInput").ap()
    imaskT = nc.dram_tensor("imaskT", [S, S], bf16, kind="ExternalInput").ap()
    wq = nc.dram_tensor("wq", [D, DH], bf16, kind="ExternalInput").ap()
    wk = nc.dram_tensor("wk", [D, DH], bf16, kind="ExternalInput").ap()
    wv = nc.dram_tensor("wv", [D, DH], bf16, kind="ExternalInput").ap()
    wo = nc.dram_tensor("wo", [DH, D], bf16, kind="ExternalInput").ap()
    out = nc.dram_tensor("out", [S, D], fp16, kind="ExternalOutput").ap()
    if DEBUG:
        dbg_qT = nc.dram_tensor("dbg_qT", [2, 128, S], bf16, kind="ExternalOutput").ap()
        dbg_kT = nc.dram_tensor("dbg_kT", [2, 128, S], bf16, kind="ExternalOutput").ap()
        dbg_vt0 = nc.dram_tensor("dbg_vt0", [128, HPC, 128], bf16, kind="ExternalOutput").ap()
        dbg_attnT = nc.dram_tensor("dbg_attnT", [2, 128, S], bf16, kind="ExternalOutput").ap()
        dbg_rden = nc.dram_tensor("dbg_rden", [64, 1024], f32, kind="ExternalOutput").ap()
        dbg_num = nc.dram_tensor("dbg_num", [64, 1024], f32, kind="ExternalOutput").ap()

    with tile.TileContext(nc) as tc:
        with tc.tile_pool(name="persist", bufs=1) as pp:
            # Persistent SBUF tiles.
            qT = [pp.tile([128, S], bf16, tag=f"qT{g}", name=f"qT{g}") for g in range(2)]
            kT = [pp.tile([128, S], bf16, tag=f"kT{g}", name=f"kT{g}") for g in range(2)]
            # vt: per 128-row k-block, per head: cols 0:64 = 1.0 (denominator
            # rows of the AV psum), cols 64:128 = V.
            vt = [pp.tile([128, HPC, 128], bf16, tag=f"v{i}", name=f"v{i}") for i in range(16)]
            wot = [pp.tile([128, D], bf16, tag=f"wo{g}", name=f"wo{g}") for g in range(2)]
            attnT = [pp.tile([128, S], bf16, tag=f"attnT{g}", name=f"attnT{g}") for g in range(2)]
            # mask for qcp0; the qcp1 tile is allocated after the x/w pool
            # frees (SBUF headroom) and prefetched during sweep 1.
            mt = [pp.tile([128, 16, 1024], bf16, tag="mask0", name="mask0"), None]
            ident = pp.tile([128, 128], f32, tag="ident", name="ident")
            identb = pp.tile([128, 128], bf16, tag="identb", name="identb")

            make_identity(nc, ident[:])
            nc.vector.tensor_copy(identb[:], ident[:])
            for st in range(16):
                nc.gpsimd.memset(vt[st][:, :, 0:DEPTH], 1.0)

            # HAM warmup so the PE is at speed when projections fire.
            with tc.tile_pool(name="psW", bufs=2, space="PSUM") as psW:
                for w in range(16):
                    psw = psW.tile([128, 128], f32, tag="warm", name="warm")
                    nc.tensor.matmul(psw[:], ident[:], ident[:],
                                     start=True, stop=True)

            imaskT_r = imaskT.rearrange("(t p) q -> p t q", p=128)

            with tc.tile_pool(name="exs", bufs=3) as exs, \
                 tc.tile_pool(name="eps", bufs=2) as eps, \
                 tc.tile_pool(name="psL", bufs=2, space="PSUM") as psL:
              with tc.tile_pool(name="xw", bufs=1) as xw:
                xt = [xw.tile([128, S], bf16, tag=f"x{d}", name=f"x{d}") for d in range(8)]
                wts = {}
                for nm in ("wq", "wk", "wv"):
                    wts[nm] = [xw.tile([128, DH], bf16, tag=f"{nm}{d}", name=f"{nm}{d}") for d in range(8)]
                # DMA priority order: xt+wq feed the q chains immediately;
                # wk is needed only when the k chains start (~18us), wv and
                # the mask only at sweep 0, wo only at the tail.
                for d in range(8):
                    nc.sync.dma_start(wts["wq"][d][:], wq[d * 128:(d + 1) * 128, :])
                    nc.sync.dma_start(xt[d][:], xT[d * 128:(d + 1) * 128, :])
                for d in range(8):
                    nc.sync.dma_start(wts["wk"][d][:], wk[d * 128:(d + 1) * 128, :])
                for d in range(8):
                    nc.sync.dma_start(wts["wv"][d][:], wv[d * 128:(d + 1) * 128, :])
                for kb in range(16):
                    nc.sync.dma_start(mt[0][:, kb:kb + 1, :],
                                      imaskT_r[:, kb:kb + 1, 0:1024])
                for g in range(2):
                    nc.sync.dma_start(wot[g][:], wo[g * 128:(g + 1) * 128, :])

                # ---- g=0 q/k projections: d-major waves over 8 psum banks,
                # riding the input DMA. ScalarE drains (it is idle here).
                with tc.tile_pool(name="psA", bufs=2, space="PSUM") as psA:
                    for wt, dst, drain in ((wts["wq"], qT, "dve"),
                                           (wts["wk"], kT, "act")):
                        chains = [(sh, psA.tile([128, 1024], f32, tag="proj", name="proj"))
                                  for sh in range(2)]
                        for d in range(8):
                            for sh, ps in chains:
                                for half in range(2):
                                    hs = slice(half * 512, (half + 1) * 512)
                                    nc.tensor.matmul(
                                        ps[:, hs],
                                        wt[d][:, 0:128],
                                        xt[d][:, sh * 1024 + half * 512:
                                               sh * 1024 + half * 512 + 512],
                                        start=(d == 0), stop=(d == 7),
                                    )
                        for sh, ps in chains:
                            if drain == "dve":
                                nc.vector.tensor_copy(dst[0][:, sh * 1024:(sh + 1) * 1024], ps[:])
                            else:
                                nc.scalar.copy(dst[0][:, sh * 1024:(sh + 1) * 1024], ps[:])

                # ---- attention: 4 sweep-offset pairs ----
                PAIRS = [(0, 0), (0, 1), (1, 0), (1, 1)]   # (qcp, g)
                emq = {}      # (pair_idx, h, kb) -> em tile
                psoq = {}     # pair_idx -> [pso_h0, pso_h1]

                def qk_exp(p, kb):
                    qcp, g = PAIRS[p]
                    # halves interleaved h0,h1,h0,h1: the two heads sit in
                    # disjoint PE row groups (rows 0:64 / 64:128), so adjacent
                    # matmuls run concurrently - half the QK wall time.
                    psl = [psL.tile([128, 1024], f32, tag="lg", name="lg")
                           for _ in range(2)]
                    for half in range(2):
                        hs = slice(half * 512, (half + 1) * 512)
                        qh = slice(qcp * 1024 + half * 512,
                                   qcp * 1024 + half * 512 + 512)
                        for h in range(2):
                            po = h * 64
                            nc.tensor.matmul(
                                psl[h][:, hs],
                                kT[g][po:po + 64, kb * 128:(kb + 1) * 128],
                                qT[g][po:po + 64, qh],
                                start=True, stop=True,
                            )
                    for h in range(2):
                        ex = exs.tile([128, 1024], bf16, tag="ex", name="ex")
                        nc.scalar.activation(ex[:], psl[h][:], EXP, scale=0.125)
                        em = exs.tile([128, 1024], bf16, tag="em", name="em", bufs=31)
                        nc.vector.tensor_tensor(em[:], ex[:], mt[qcp][:, kb, :], MULT)
                        emq[(p, h, kb)] = em

                def av(p, kb, psO, heads=(0, 1)):
                    qcp, g = PAIRS[p]
                    if kb == 0:
                        psoq.setdefault(p, [None, None])
                        for h in heads:
                            psoq[p][h] = psO.tile([128, 1024], f32, tag="av",
                                                  name=f"av{h}")
                    for h in heads:
                        for half in range(2):
                            hs = slice(half * 512, (half + 1) * 512)
                            nc.tensor.matmul(
                                psoq[p][h][:, hs],
                                vt[kb][:, 2 * g + h, :],
                                emq[(p, h, kb)][:, hs],
                                start=(kb == 0), stop=(kb == 15),
                            )
                        if kb == 15:
                            for kk in range(16):
                                del emq[(p, h, kk)]

                def epilogue(p, heads=(0, 1)):
                    qcp, g = PAIRS[p]
                    qs = slice(qcp * 1024, (qcp + 1) * 1024)
                    for h in heads:
                        po = h * 64
                        pso = psoq[p][h]
                        rden = eps.tile([64, 1024], f32, tag="rden", name="rden")
                        nc.vector.reciprocal_approx_fast(rden[:], pso[0:64, :])
                        if DEBUG and p == 0 and h == 0:
                            num_s = eps.tile([64, 1024], f32, tag="nums", name="nums")
                            nc.vector.tensor_copy(num_s[:], pso[64:128, :])
                            nc.sync.dma_start(dbg_num[:], num_s[:])
                            nc.sync.dma_start(dbg_rden[:], rden[:])
                        nc.vector.tensor_tensor(
                            attnT[g][po:po + 64, qs],
                            pso[64:128, :], rden[:], MULT,
                        )

                # Sweep 0: pair 0 QK/exp with v-proj + g=1 q/k proj as PE
                # fillers (no AVs yet -> psum banks 5-8 are free for them).
                with tc.tile_pool(name="psV", bufs=2, space="PSUM") as psV, \
                     tc.tile_pool(name="psA2", bufs=2, space="PSUM") as psA2:
                    g1chains = []
                    for wt, dst in ((wts["wq"], qT), (wts["wk"], kT)):
                        for sc in range(4):
                            g1chains.append((wt, dst, sc))
                    g1state = {}

                    def g1_step(kb):
                        # half a chain (4 d-matmuls) per iteration
                        ci = kb // 2
                        if ci >= len(g1chains):
                            return
                        wt, dst, sc = g1chains[ci]
                        if kb % 2 == 0:
                            g1state[ci] = psA2.tile([128, 512], f32, tag="g1", name="g1")
                            for d in range(4):
                                nc.tensor.matmul(
                                    g1state[ci][:],
                                    wt[d][:, 128:256],
                                    xt[d][:, sc * 512:(sc + 1) * 512],
                                    start=(d == 0), stop=False,
                                )
                        else:
                            for d in range(4, 8):
                                nc.tensor.matmul(
                                    g1state[ci][:],
                                    wt[d][:, 128:256],
                                    xt[d][:, sc * 512:(sc + 1) * 512],
                                    start=False, stop=(d == 7),
                                )
                            nc.vector.tensor_copy(
                                dst[1][:, sc * 512:(sc + 1) * 512], g1state[ci][:])

                    def v_chain(st):
                        ps = psV.tile([128, DH], f32, tag="vproj", name="vproj")
                        for d in range(8):
                            nc.tensor.matmul(
                                ps[:],
                                xt[d][:, st * 128:(st + 1) * 128],
                                wts["wv"][d][:],
                                start=(d == 0), stop=(d == 7),
                            )
                        nc.vector.tensor_copy(
                            vt[st][:, :, DEPTH:128],
                            ps[:].rearrange("p (h e) -> p h e", h=HPC),
                        )

                    for kb in range(16):
                        qk_exp(0, kb)
                        v_chain(kb)
                        g1_step(kb)

              # x/w pool closed: allocate the qcp1 mask tile and prefetch it
              # during sweep 1.
              with tc.tile_pool(name="mt1", bufs=1) as mp:
                mt[1] = mp.tile([128, 16, 1024], bf16, tag="mask1", name="mask1")
                for kb in range(16):
                    nc.sync.dma_start(mt[1][:, kb:kb + 1, :],
                                      imaskT_r[:, kb:kb + 1, 1024:2048])

                # Sweeps 1-3 + final AV burst.
                with tc.tile_pool(name="psO", bufs=2, space="PSUM") as psO:
                    for p in range(1, 4):
                        for _ in range(4):
                            nc.tensor.ldweights(identb[:])
                        for kb in range(16):
                            qk_exp(p, kb)
                            av(p - 1, kb, psO)
                        epilogue(p - 1)
                    # Final burst, head-serial: h0's AVs start as soon as the
                    # previous pair's first pso slot frees; h0's epilogue then
                    # overlaps h1's AV burst.
                    for kb in range(16):
                        av(3, kb, psO, heads=(0,))
                    epilogue(3, heads=(0,))
                    for kb in range(16):
                        av(3, kb, psO, heads=(1,))
                    epilogue(3, heads=(1,))

            if DEBUG:
                for g in range(2):
                    nc.sync.dma_start(dbg_qT[g], qT[g][:])
                    nc.sync.dma_start(dbg_kT[g], kT[g][:])
                    nc.sync.dma_start(dbg_attnT[g], attnT[g][:])
                nc.sync.dma_start(dbg_vt0[:], vt[0][:])

            # ---- output projection tail (fp16 out) ----
            with tc.tile_pool(name="ot", bufs=3) as ob, \
                 tc.tile_pool(name="psF", bufs=2, space="PSUM") as psF:
                for st in range(16):
                    psf = psF.tile([128, D], f32, tag="po", name="po")
                    for nch in range(2):
                        hs = slice(nch * 512, (nch + 1) * 512)
                        for g in range(2):
                            nc.tensor.matmul(
                                psf[:, hs],
                                attnT[g][:, st * 128:(st + 1) * 128],
                                wot[g][:, hs],
                                start=(g == 0), stop=(g == 1),
                            )
                    ot = ob.tile([128, D], fp16, tag="otile", name="otile")
                    if st % 2 == 0:
                        nc.vector.tensor_copy(ot[:], psf[:])
                    else:
                        nc.scalar.copy(ot[:], psf[:])
                    nc.sync.dma_start(out[st * 128:(st + 1) * 128, :], ot[:])

    nc.compile()
    nc.m = get_hw_module(nc.m)
    return nc


def _get_program():
    if "nc" not in _CACHE:
        _CACHE["nc"] = _build_program()
    return _CACHE["nc"]


def _make_in_maps(query, attention_mask, Wq, Wk, Wv, Wo):
    import ml_dtypes

    bf16 = ml_dtypes.bfloat16
    in_maps = []
    imaskT_b = []
    xT_b = []
    for b in range(B):
        imaskT_b.append(
            np.ascontiguousarray(1 - attention_mask[b, 0].T).astype(bf16)
        )
        xT_b.append(np.ascontiguousarray(query[b].T.astype(bf16)))
    for c in range(CORES):
        b, hg = c // HG, c % HG
        cs = slice(hg * DH, (hg + 1) * DH)
        in_maps.append({
            "xT": xT_b[b],
            "imaskT": imaskT_b[b],
            "wq": np.ascontiguousarray(Wq[:, cs].astype(bf16)),
            "wk": np.ascontiguousarray(Wk[:, cs].astype(bf16)),
            "wv": np.ascontiguousarray(Wv[:, cs].astype(bf16)),
            "wo": np.ascontiguousarray(Wo[cs, :].astype(bf16)),
        })
    return in_maps


def _run(inputs, trace=False):
    from concourse.bass_utils import run_bass_kernel_spmd

    nc = _get_program()
    in_maps = _make_in_maps(**inputs)
    res = run_bass_kernel_spmd(
        nc, in_maps, core_ids=list(range(CORES)), trace=trace,
    )
    outs = [res.results[c]["out"].astype(np.float32) for c in range(CORES)]
    full = np.empty((B, S, D), dtype=np.float32)
    for b in range(B):
        acc = outs[4 * b]
        for hg in range(1, HG):
            acc = acc + outs[4 * b + hg]
        full[b] = acc
    return full, res


def kernel(query, attention_mask, Wq, Wk, Wv, Wo):
    full, _ = _run(dict(
        query=np.asarray(query), attention_mask=np.asarray(attention_mask),
        Wq=np.asarray(Wq), Wk=np.asarray(Wk), Wv=np.asarray(Wv),
        Wo=np.asarray(Wo),
    ))
    return full
